# revision 1
# baseline (speedup 1.0000x reference)
"""Trainium2 Bass kernel for nn_ExtendedSelfAttention (B=4, S=2048, D=4096, H=1).

With n_heads=1 the softmax is over a size-1 axis, so attention weights are
exactly 1.0 and the module reduces to:

    out = (value @ Wv.T + bv) @ Wo.T + bo
        = value @ (Wo @ Wv).T + (Wo @ bv + bo)

(query/key/Wq/Wk never affect the output.) Since there are 8192 tokens but
only 4096 features, composing the weights first cuts total FLOPs by 25%:
computing Wc^T = (Wo @ Wv)^T costs one 4096^3 GEMM (sharded 8 ways), after
which only ONE token GEMM is needed instead of two.

Sharding (no collectives):
  phase A: core c computes Wc^T[:, c*512:(c+1)*512]   (1024 matmuls)
           lhsT = Wv[f-tile, k-block] (natural layout), rhs = Wo^T slice
  phase B: core c computes out[:, c*512:(c+1)*512] for ALL 8192 tokens
           lhsT = x^T tiles, rhs = Wc^T slice (SBUF-resident)  (2048 matmuls)
Output is column-sharded; the host concatenates. The fused bias
bias2 = Wo @ bv + bo is computed exactly on the host and added in phase B.

Compute dtype bf16 (host-cast), fp32 PSUM accumulation, fp32 output.
"""

import numpy as np

B, S, D = 4, 2048, 4096
N_CORES = 8
TOK = B * S           # 8192 tokens
P = 128
KO = D // P           # 32 contraction tiles
GBLK = D // N_CORES   # 512 output columns per core
TT = TOK // P         # 64 token tiles

_CACHED = {}


def _build_nc():
    import concourse.bass as bass  # noqa: F401  (registers engine builders)
    import concourse.tile as tile
    from concourse import bacc, mybir

    bf16 = mybir.dt.bfloat16
    f32 = mybir.dt.float32

    nc = bacc.Bacc("TRN2", target_bir_lowering=False, debug=False,
                   num_devices=N_CORES)

    # wv[m, p, fo, c2] = Wv[fo*128+p, m*128+c2]   (lhsT tiles for phase A)
    wv = nc.declare_dram_parameter("wv", [KO, P, KO, P], bf16, isOutput=False)
    # woT[p, fo, g] = Wo[cg0+g, fo*128+p]          (rhs for phase A, per-core)
    woT = nc.declare_dram_parameter("woT", [P, KO, GBLK], bf16, isOutput=False)
    # xt[tt, p, ko, tc] = x[tt*128+tc, ko*128+p]   (lhsT tiles for phase B)
    xt = nc.declare_dram_parameter("xt", [TT, P, KO, P], bf16, isOutput=False)
    b2 = nc.declare_dram_parameter("b2", [P, GBLK], f32, isOutput=False)
    out = nc.declare_dram_parameter("out", [TOK, GBLK], f32, isOutput=True)

    with tile.TileContext(nc) as tc:
        with tc.tile_pool(name="const", bufs=1) as const_pool, \
             tc.tile_pool(name="wot", bufs=1) as wot_pool, \
             tc.tile_pool(name="wct", bufs=1) as wct_pool, \
             tc.tile_pool(name="wvp", bufs=4) as wv_pool, \
             tc.tile_pool(name="xtp", bufs=4) as xt_pool, \
             tc.tile_pool(name="psum", bufs=8, space="PSUM") as psum_pool, \
             tc.tile_pool(name="stage", bufs=4) as stage_pool:
            wot_sb = wot_pool.tile([P, KO, GBLK], bf16)
            wct_sb = wct_pool.tile([P, KO, GBLK], bf16)

            # Prewarm the PE during the otherwise-idle DMA ramp (~14us): the
            # HAM clock gate needs ~3.4us of sustained matmul activity to
            # lift the PE from 1.2 to 2.4 GHz, so run one long dummy
            # accumulation group on memset data. Sized to end just before
            # the first real weights land (longer idle re-throttles).
            warm_lhs = const_pool.tile([P, P], bf16, tag="warm_lhs")
            warm_rhs = const_pool.tile([P, GBLK], bf16, tag="warm_rhs")
            nc.vector.memset(warm_lhs[:], 0.0)
            nc.vector.memset(warm_rhs[:], 0.0)
            N_WARM = 26
            dps = psum_pool.tile([P, GBLK], f32, tag="ps")
            for i in range(N_WARM):
                nc.tensor.matmul(dps[:], warm_lhs[:], warm_rhs[:],
                                 start=(i == 0), stop=(i == N_WARM - 1))

            # ---- phase A: Wc^T slice = Wv.T-contracted with Wo^T slice ----
            # Startup: interleave wot 8-ftile chunks (8KB/partition descriptors
            # -> full DMA rate) with the first wv tiles so the first matmul
            # group can start ~5us in and never starves afterwards.
            wv_pre = []
            nc.sync.dma_start(out=wot_sb[:, 0:8, :], in_=woT[:, 0:8, :])
            wv_t = wv_pool.tile([P, KO, P], bf16, tag="wv")
            nc.sync.dma_start(out=wv_t[:], in_=wv[0])
            wv_pre.append(wv_t)
            for g in range(1, 4):
                nc.sync.dma_start(out=wot_sb[:, g * 8:(g + 1) * 8, :],
                                  in_=woT[:, g * 8:(g + 1) * 8, :])
            for m in range(1, 3):
                wv_t = wv_pool.tile([P, KO, P], bf16, tag="wv")
                nc.sync.dma_start(out=wv_t[:], in_=wv[m])
                wv_pre.append(wv_t)

            b2_t = const_pool.tile([P, GBLK], f32)
            nc.sync.dma_start(out=b2_t[:], in_=b2[:])

            for mA in range(KO):
                if mA < 3:
                    wv_t = wv_pre[mA]
                else:
                    wv_t = wv_pool.tile([P, KO, P], bf16, tag="wv")
                    nc.sync.dma_start(out=wv_t[:], in_=wv[mA])
                ps = psum_pool.tile([P, GBLK], f32)
                for fA in range(KO):
                    nc.tensor.matmul(
                        ps[:], wv_t[:, fA, :], wot_sb[:, fA, :],
                        start=(fA == 0), stop=(fA == KO - 1),
                    )
                nc.vector.tensor_copy(wct_sb[:, mA, :], ps[:])

            # ---- phase B: out slice = x @ Wc^T slice (+ bias2) ----
            for tt in range(TT):
                xt_t = xt_pool.tile([P, KO, P], bf16)
                nc.sync.dma_start(out=xt_t[:], in_=xt[tt])
                ps = psum_pool.tile([P, GBLK], f32)
                for k in range(KO):
                    nc.tensor.matmul(
                        ps[:], xt_t[:, k, :], wct_sb[:, k, :],
                        start=(k == 0), stop=(k == KO - 1),
                    )
                st = stage_pool.tile([P, GBLK], f32)
                nc.vector.tensor_add(st[:], ps[:], b2_t[:])
                nc.sync.dma_start(
                    out=out[tt * P:(tt + 1) * P, :], in_=st[:])
    nc.compile()
    return nc


def _get_nc():
    if "nc" not in _CACHED:
        _CACHED["nc"] = _build_nc()
    return _CACHED["nc"]


def _prep_inputs(value, Wv, bv, Wo, bo):
    import ml_dtypes
    bf16 = ml_dtypes.bfloat16

    x = np.asarray(value, np.float32).reshape(TOK, D)
    Wv = np.asarray(Wv, np.float32)
    Wo = np.asarray(Wo, np.float32)
    bv = np.asarray(bv, np.float32)
    bo = np.asarray(bo, np.float32)

    # xt[tt, p, ko, tc] = x[tt*128+tc, ko*128+p]
    xt = np.ascontiguousarray(
        x.reshape(TT, P, KO, P).transpose(0, 3, 2, 1)).astype(bf16)
    # wv_p[m, p, fo, c2] = Wv[fo*128+p, m*128+c2]
    wv_p = np.ascontiguousarray(
        Wv.reshape(KO, P, KO, P).transpose(2, 1, 0, 3)).astype(bf16)
    # woT_full[c][p, fo, g] = Wo[c*GBLK+g, fo*128+p]
    woT_full = Wo.reshape(N_CORES, GBLK, KO, P).transpose(0, 3, 2, 1)

    bias2 = (Wo.astype(np.float64) @ bv.astype(np.float64)
             + bo.astype(np.float64)).astype(np.float32)

    in_maps = []
    for c in range(N_CORES):
        b2_c = np.ascontiguousarray(np.broadcast_to(
            bias2[c * GBLK:(c + 1) * GBLK][None, :], (P, GBLK)))
        in_maps.append({
            "xt": xt,
            "wv": wv_p,
            "woT": np.ascontiguousarray(woT_full[c]).astype(bf16),
            "b2": b2_c,
        })
    return in_maps


def _run(in_maps, trace=False):
    from concourse.bass_utils import run_bass_kernel_spmd
    nc = _get_nc()
    res = run_bass_kernel_spmd(nc, in_maps, list(range(N_CORES)), trace=trace)
    return res


def kernel(**inputs):
    in_maps = _prep_inputs(inputs["value"], inputs["Wv"], inputs["bv"],
                           inputs["Wo"], inputs["bo"])
    res = _run(in_maps, trace=False)
    out = np.empty((TOK, D), np.float32)
    for c in range(N_CORES):
        out[:, c * GBLK:(c + 1) * GBLK] = res.results[c]["out"]
    return out.reshape(B, S, D)



# revision 6
# speedup vs baseline: 1.0823x; 1.0823x over previous
"""Trainium2 Bass kernel for nn_ExtendedSelfAttention (B=4, S=2048, D=4096, H=1).

With n_heads=1 the softmax is over a size-1 axis, so attention weights are
exactly 1.0 and the module reduces to:

    out = (value @ Wv.T + bv) @ Wo.T + bo
        = value @ (Wo @ Wv).T + (Wo @ bv + bo)

(query/key/Wq/Wk never affect the output.) Composing the weights first cuts
total FLOPs by 25%: Wc^T = (Wo @ Wv)^T costs one 4096^3 GEMM (sharded 8
ways), after which only ONE token GEMM is needed.

Sharding (no collectives): core c computes output columns [c*512,(c+1)*512)
for all 8192 tokens; host concatenates.

Phase A (classical): Wc^T slice via 1024 bf16 matmuls (N=512).
Phase B (Strassen-Winograd, 1 level): out = X @ Wc^T-slice with
  M = 8192 tokens -> 2x4096, K = 4096 -> 2x2048, N = 512 -> 2x256.
  7 products of [4096x2048]@[2048x256] instead of 8: PE work x7/8.
  Per token-tile pair (t, t+32): 4 S-pre-adds (bf16, DVE), 7x16 matmuls
  (N=256), 11 U-combines (fp32, DVE). T-operands (Wc-side combos) are
  built once on the DVE, interleaved with phase A's back half.
  Winograd form: S1=A21+A22, S2=S1-A11, S3=A11-A21, S4=A12-S2;
  T1=B12-B11, T2=B22-T1, T3=B22-B12, T4=T2-B21;
  M1=A11*B11, M2=A12*B21, M3=S4*B22, M4=A22*T4, M5=S1*T1, M6=S2*T2,
  M7=S3*T3; C11=M1+M2, C12=U4+M3, C21=U3-M4, C22=U3+M5
  with U2=M1+M6, U3=U2+M7, U4=U2+M5.

Compute dtype bf16 (host-cast), fp32 PSUM accumulation, fp32 output.
"""

import numpy as np

B, S, D = 4, 2048, 4096
N_CORES = 8
TOK = B * S           # 8192 tokens
P = 128
KO = D // P           # 32 contraction tiles
KH = KO // 2          # 16 tiles per Strassen K-half
GBLK = D // N_CORES   # 512 output columns per core
GB2 = GBLK // 2       # 256 per Strassen N-half
TT = TOK // P         # 64 token tiles
NPAIR = TT // 2       # 32 Strassen token-tile pairs

_CACHED = {}


def _build_nc():
    import concourse.bass as bass  # noqa: F401  (registers engine builders)
    import concourse.tile as tile
    from concourse import bacc, mybir

    bf16 = mybir.dt.bfloat16
    f32 = mybir.dt.float32

    nc = bacc.Bacc("TRN2", target_bir_lowering=False, debug=False,
                   num_devices=N_CORES)

    # wv[m, p, fo, c2] = Wv[fo*128+p, m*128+c2]   (lhsT tiles for phase A)
    wv = nc.declare_dram_parameter("wv", [KO, P, KO, P], bf16, isOutput=False)
    # woT[p, fo, g] = Wo[cg0+g, fo*128+p]          (rhs for phase A, per-core)
    woT = nc.declare_dram_parameter("woT", [P, KO, GBLK], bf16, isOutput=False)
    # xt[tt, p, ko, tc] = x[tt*128+tc, ko*128+p]   (lhsT tiles for phase B)
    xt = nc.declare_dram_parameter("xt", [TT, P, KO, P], bf16, isOutput=False)
    b2 = nc.declare_dram_parameter("b2", [P, GBLK], f32, isOutput=False)
    out = nc.declare_dram_parameter("out", [TOK, GBLK], f32, isOutput=True)

    with tile.TileContext(nc) as tc:
        with tc.tile_pool(name="const", bufs=1) as const_pool, \
             tc.tile_pool(name="wot", bufs=1) as wot_pool, \
             tc.tile_pool(name="wct", bufs=1) as wct_pool, \
             tc.tile_pool(name="shared", bufs=6) as sh_pool, \
             tc.tile_pool(name="ao", bufs=8) as ao_pool, \
             tc.tile_pool(name="bo", bufs=1) as bo_pool, \
             tc.tile_pool(name="psum", bufs=8, space="PSUM") as psum_pool, \
             tc.tile_pool(name="ustage", bufs=6) as ust_pool, \
             tc.tile_pool(name="outst", bufs=8) as ost_pool:
            wot_sb = wot_pool.tile([P, KO, GBLK], bf16)
            wct_sb = wct_pool.tile([P, KO, GBLK], bf16)
            # T-operands for phase B (Wc-side Winograd combos)
            t1_sb = bo_pool.tile([P, KH, GB2], bf16, tag="t1")
            t2_sb = bo_pool.tile([P, KH, GB2], bf16, tag="t2")
            t3_sb = bo_pool.tile([P, KH, GB2], bf16, tag="t3")
            t4_sb = bo_pool.tile([P, KH, GB2], bf16, tag="t4")
            t_sb = [t1_sb, t2_sb, t3_sb, t4_sb]

            # Prewarm the PE during the otherwise-idle DMA ramp: the HAM
            # clock gate needs ~3.4us of sustained matmul activity to lift
            # the PE from 1.2 to 2.4 GHz.  Inputs are uninitialized SBUF
            # (wct garbage) - values are irrelevant, the psum is discarded.
            warm_lhs = wct_sb[:, 0, 0:P]
            warm_rhs = wct_sb[:, 1, :]
            N_WARM = 26
            dps = psum_pool.tile([P, GBLK], f32, tag="ps")
            for i in range(N_WARM):
                nc.tensor.matmul(dps[:], warm_lhs, warm_rhs,
                                 start=(i == 0), stop=(i == N_WARM - 1))

            # ---- phase A startup DMA ladder (interleaved wot/wv) ----
            wv_pre = []
            nc.sync.dma_start(out=wot_sb[:, 0:8, :], in_=woT[:, 0:8, :])
            wv_t = sh_pool.tile([P, KO, P], bf16, tag="sh")
            nc.sync.dma_start(out=wv_t[:], in_=wv[0])
            wv_pre.append(wv_t)
            for g in range(1, 4):
                nc.sync.dma_start(out=wot_sb[:, g * 8:(g + 1) * 8, :],
                                  in_=woT[:, g * 8:(g + 1) * 8, :])
                if g < 3:
                    wv_t = sh_pool.tile([P, KO, P], bf16, tag="sh")
                    nc.sync.dma_start(out=wv_t[:], in_=wv[g])
                    wv_pre.append(wv_t)

            b2_t = const_pool.tile([P, GBLK], f32)
            nc.sync.dma_start(out=b2_t[:], in_=b2[:])

            xt_tiles = {}   # pair idx -> (xt1, xt2)

            def prefetch_pair(j):
                x1 = sh_pool.tile([P, KO, P], bf16, tag="sh")
                nc.sync.dma_start(out=x1[:], in_=xt[j])
                x2 = sh_pool.tile([P, KO, P], bf16, tag="sh")
                nc.sync.dma_start(out=x2[:], in_=xt[j + NPAIR])
                xt_tiles[j] = (x1, x2)

            s_tiles = {}    # pair idx -> [S1, S2, S3, S4]

            def emit_s(j):
                x1, x2 = xt_tiles[j]
                a11 = x1[:, 0:KH, :]
                a12 = x1[:, KH:KO, :]
                a21 = x2[:, 0:KH, :]
                a22 = x2[:, KH:KO, :]
                s1 = ao_pool.tile([P, KH, P], bf16, tag="s")
                nc.vector.tensor_add(s1[:], a21, a22)
                s2 = ao_pool.tile([P, KH, P], bf16, tag="s")
                nc.vector.tensor_sub(s2[:], s1[:], a11)
                s3 = ao_pool.tile([P, KH, P], bf16, tag="s")
                nc.vector.tensor_sub(s3[:], a11, a21)
                s4 = ao_pool.tile([P, KH, P], bf16, tag="s")
                nc.vector.tensor_sub(s4[:], a12, s2[:])
                s_tiles[j] = [s1, s2, s3, s4]

            # ---- phase A: Wc^T slice = Wv.T-contracted with Wo^T slice ----
            for mA in range(KO):
                if mA < 3:
                    wv_t = wv_pre[mA]
                else:
                    wv_t = sh_pool.tile([P, KO, P], bf16, tag="sh")
                    nc.sync.dma_start(out=wv_t[:], in_=wv[mA])
                ps = psum_pool.tile([P, GBLK], f32, tag="ps")
                for fA in range(KO):
                    nc.tensor.matmul(
                        ps[:], wv_t[:, fA, :], wot_sb[:, fA, :],
                        start=(fA == 0), stop=(fA == KO - 1),
                    )
                nc.vector.tensor_copy(wct_sb[:, mA, :], ps[:])
                # Build T-operands once both wct K-halves for ko are cast.
                if mA >= KH:
                    ko = mA - KH
                    b11 = wct_sb[:, ko, 0:GB2]
                    b12 = wct_sb[:, ko, GB2:GBLK]
                    b21 = wct_sb[:, mA, 0:GB2]
                    b22 = wct_sb[:, mA, GB2:GBLK]
                    nc.vector.tensor_sub(t_sb[0][:, ko, :], b12, b11)  # T1
                    nc.vector.tensor_sub(t_sb[1][:, ko, :], b22,
                                         t_sb[0][:, ko, :])            # T2
                    nc.vector.tensor_sub(t_sb[2][:, ko, :], b22, b12)  # T3
                    nc.vector.tensor_sub(t_sb[3][:, ko, :],
                                         t_sb[1][:, ko, :], b21)       # T4
                # Prefetch phase B inputs late in phase A (off the DMA
                # critical path at startup, early enough to hide latency).
                if mA == 8:
                    prefetch_pair(0)
                elif mA == 12:
                    emit_s(0)
                elif mA == 22:
                    prefetch_pair(1)
                elif mA == 29:
                    emit_s(1)

            # ---- phase B: Strassen-Winograd token GEMM ----
            b2L = b2_t[:, 0:GB2]
            b2R = b2_t[:, GB2:GBLK]
            for j in range(NPAIR):
                if j + 2 < NPAIR:
                    prefetch_pair(j + 2)
                if j + 1 < NPAIR and j + 1 >= 2:
                    emit_s(j + 1)
                x1, x2 = xt_tiles.pop(j)
                s1, s2, s3, s4 = s_tiles.pop(j)
                a11 = x1[:, 0:KH, :]
                a12 = x1[:, KH:KO, :]
                a22 = x2[:, KH:KO, :]
                # psum packing: [M1|M2], [M3|M4], [M5|M6], [M7|-]
                ps12 = psum_pool.tile([P, GBLK], f32, tag="ps")
                ps34 = psum_pool.tile([P, GBLK], f32, tag="ps")
                ps56 = psum_pool.tile([P, GBLK], f32, tag="ps")
                ps7 = psum_pool.tile([P, GBLK], f32, tag="ps")
                m1, m2 = ps12[:, 0:GB2], ps12[:, GB2:GBLK]
                m3, m4 = ps34[:, 0:GB2], ps34[:, GB2:GBLK]
                m5, m6 = ps56[:, 0:GB2], ps56[:, GB2:GBLK]
                m7 = ps7[:, 0:GB2]
                # (dst, lhsT(ko), rhs(ko)) per product; M3 last (S4 is the
                # deepest S-chain), plain-operand products first.
                prods = [
                    (m1, lambda k: x1[:, k, :],
                     lambda k: wct_sb[:, k, 0:GB2]),            # A11*B11
                    (m2, lambda k: x1[:, KH + k, :],
                     lambda k: wct_sb[:, KH + k, 0:GB2]),       # A12*B21
                    (m4, lambda k: x2[:, KH + k, :],
                     lambda k: t_sb[3][:, k, :]),               # A22*T4
                    (m5, lambda k: s1[:, k, :],
                     lambda k: t_sb[0][:, k, :]),               # S1*T1
                    (m6, lambda k: s2[:, k, :],
                     lambda k: t_sb[1][:, k, :]),               # S2*T2
                    (m7, lambda k: s3[:, k, :],
                     lambda k: t_sb[2][:, k, :]),               # S3*T3
                    (m3, lambda k: s4[:, k, :],
                     lambda k: wct_sb[:, KH + k, GB2:GBLK]),    # S4*B22
                ]
                for dst, lf, rf in prods:
                    for k in range(KH):
                        nc.tensor.matmul(dst, lf(k), rf(k),
                                         start=(k == 0), stop=(k == KH - 1))
                # U-combines -> output pieces (+bias), then store.
                t0 = j * P
                t1r = (j + NPAIR) * P
                # DVE may read only ONE operand from PSUM per op: stage M1
                # into SBUF on the (otherwise idle) scalar engine, and fold
                # the C11 bias add so every DVE op has <=1 PSUM input.
                m1s = ust_pool.tile([P, GB2], f32, tag="u")
                nc.scalar.copy(m1s[:], m1)
                z1 = ust_pool.tile([P, GB2], f32, tag="u")
                nc.vector.tensor_add(z1[:], m2, b2L)
                o1 = ost_pool.tile([P, GB2], f32, tag="o")
                nc.vector.tensor_add(o1[:], z1[:], m1)
                nc.sync.dma_start(out=out[t0:t0 + P, 0:GB2], in_=o1[:])
                u2 = ust_pool.tile([P, GB2], f32, tag="u")
                nc.vector.tensor_add(u2[:], m1s[:], m6)
                u3 = ust_pool.tile([P, GB2], f32, tag="u")
                nc.vector.tensor_add(u3[:], u2[:], m7)
                u4 = ust_pool.tile([P, GB2], f32, tag="u")
                nc.vector.tensor_add(u4[:], u2[:], m5)
                u5 = ust_pool.tile([P, GB2], f32, tag="u")
                nc.vector.tensor_add(u5[:], u4[:], m3)
                o2 = ost_pool.tile([P, GB2], f32, tag="o")
                nc.vector.tensor_add(o2[:], u5[:], b2R)
                nc.sync.dma_start(out=out[t0:t0 + P, GB2:GBLK], in_=o2[:])
                u6 = ust_pool.tile([P, GB2], f32, tag="u")
                nc.vector.tensor_sub(u6[:], u3[:], m4)
                o3 = ost_pool.tile([P, GB2], f32, tag="o")
                nc.vector.tensor_add(o3[:], u6[:], b2L)
                nc.sync.dma_start(out=out[t1r:t1r + P, 0:GB2], in_=o3[:])
                u7 = ust_pool.tile([P, GB2], f32, tag="u")
                nc.vector.tensor_add(u7[:], u3[:], m5)
                o4 = ost_pool.tile([P, GB2], f32, tag="o")
                nc.vector.tensor_add(o4[:], u7[:], b2R)
                nc.sync.dma_start(out=out[t1r:t1r + P, GB2:GBLK], in_=o4[:])
    nc.compile()
    return nc


def _get_nc():
    if "nc" not in _CACHED:
        _CACHED["nc"] = _build_nc()
    return _CACHED["nc"]


def _prep_inputs(value, Wv, bv, Wo, bo):
    import ml_dtypes
    bf16 = ml_dtypes.bfloat16

    x = np.asarray(value, np.float32).reshape(TOK, D)
    Wv = np.asarray(Wv, np.float32)
    Wo = np.asarray(Wo, np.float32)
    bv = np.asarray(bv, np.float32)
    bo = np.asarray(bo, np.float32)

    # xt[tt, p, ko, tc] = x[tt*128+tc, ko*128+p]
    xt = np.ascontiguousarray(
        x.reshape(TT, P, KO, P).transpose(0, 3, 2, 1)).astype(bf16)
    # wv_p[m, p, fo, c2] = Wv[fo*128+p, m*128+c2]
    wv_p = np.ascontiguousarray(
        Wv.reshape(KO, P, KO, P).transpose(2, 1, 0, 3)).astype(bf16)
    # woT_full[c][p, fo, g] = Wo[c*GBLK+g, fo*128+p]
    woT_full = Wo.reshape(N_CORES, GBLK, KO, P).transpose(0, 3, 2, 1)

    bias2 = (Wo.astype(np.float64) @ bv.astype(np.float64)
             + bo.astype(np.float64)).astype(np.float32)

    in_maps = []
    for c in range(N_CORES):
        b2_c = np.ascontiguousarray(np.broadcast_to(
            bias2[c * GBLK:(c + 1) * GBLK][None, :], (P, GBLK)))
        in_maps.append({
            "xt": xt,
            "wv": wv_p,
            "woT": np.ascontiguousarray(woT_full[c]).astype(bf16),
            "b2": b2_c,
        })
    return in_maps


def _run(in_maps, trace=False):
    from concourse.bass_utils import run_bass_kernel_spmd
    nc = _get_nc()
    res = run_bass_kernel_spmd(nc, in_maps, list(range(N_CORES)), trace=trace)
    return res


def kernel(**inputs):
    in_maps = _prep_inputs(inputs["value"], inputs["Wv"], inputs["bv"],
                           inputs["Wo"], inputs["bo"])
    res = _run(in_maps, trace=False)
    out = np.empty((TOK, D), np.float32)
    for c in range(N_CORES):
        out[:, c * GBLK:(c + 1) * GBLK] = res.results[c]["out"]
    return out.reshape(B, S, D)
